# revision 12
# baseline (speedup 1.0000x reference)
"""Trainium2 Bass kernel: ActionEmbedder (1x1 conv on spatially-tiled action).

y[b,e] = relu(sum_a action[b,a] * conv_w[e,a] + conv_b[e])
out[b,e,h,w] = y[b,e]  (broadcast over 64x64 spatial positions)

Sharding: data-parallel over batch B=128 across 8 cores (16 rows each);
conv_w/conv_b replicated. Each core computes its 16x256 y block with 4
matmuls, then broadcasts it into [16*256, 4096] rows and streams 64 MiB
to HBM — the kernel is HBM-write-bandwidth bound.
"""

import os
import sys

import numpy as np

B, A, E, H, W = 128, 256, 256, 64, 64
NCORES = 8
BC = B // NCORES  # 16 batch rows per core
HW = H * W  # 4096 spatial positions
ROWS = BC * E  # 4096 output rows per core, each HW f32 long
TILE_F = 2 * HW  # fill-tile free dim: one batch row (= 2 e-halves) per tile


def _ensure_import_path():
    try:
        import concourse.bass  # noqa: F401
    except ImportError:
        for p in ("/opt/trn_rl_repo", os.path.expanduser("~/.axon_site/_ro/trn_rl_repo")):
            if os.path.isdir(p) and p not in sys.path:
                sys.path.insert(0, p)
        import concourse.bass  # noqa: F401


_NC = None


def _build():
    """Build (once) the single-core SPMD Bass program."""
    global _NC
    if _NC is not None:
        return _NC
    _ensure_import_path()
    import concourse.bacc as bacc
    import concourse.mybir as mybir
    import concourse.tile as tile

    fp32 = mybir.dt.float32
    # Bacc (not plain Bass): its compile() runs generate_event_semaphores,
    # which splits multi-wait instructions into EventSemaphore + inst — the
    # TRN2 ISA allows at most one sync wait per regular instruction.
    nc = bacc.Bacc("TRN2", target_bir_lowering=False, debug=False, num_devices=NCORES)

    # All per-core inputs packed into one [128, 546] tensor (single DMA, so
    # downstream matmuls wait on a single DMA semaphore — the PE instruction
    # has very few sync-wait slots). E is permuted even/odd on the host so
    # that partition p ends up holding y[., e=2p+j] for parity j — then each
    # partition's two output rows per batch block (2p, 2p+1) are CONTIGUOUS
    # 32KB in DRAM, halving DMA descriptor count vs the identity layout.
    # Host-side layout along the free dim ((i, j) = (A-chunk, E-parity)):
    #   [(2i+j)*128 : (2i+j+1)*128)  lhsT(i,j)[p, m] = conv_w[2m+j, 128i+p]
    #   [512:528)   actT chunk0 act0[p, b] = action[b, p]
    #   [528:544)   actT chunk1 act1[p, b] = action[b, 128 + p]
    #   [544]       bias_j=0[p] = conv_b[2p]
    #   [545]       bias_j=1[p] = conv_b[2p + 1]
    F_PACKED = 2 * E + 2 * BC + 2
    packed = nc.dram_tensor("packed", [128, F_PACKED], fp32, kind="ExternalInput")
    out = nc.dram_tensor("out", [ROWS, HW], fp32, kind="ExternalOutput")

    with tile.TileContext(nc) as tc:
        with (
            tc.tile_pool(name="const", bufs=1) as cpool,
            tc.tile_pool(name="psum", bufs=1, space="PSUM") as ppool,
            tc.tile_pool(name="fill", bufs=4) as fpool,
        ):
            pk = cpool.tile([128, F_PACKED], fp32, name="pk", tag="pk")
            nc.sync.dma_start(pk[:], packed[:])

            # --- yT[e,b] = relu(w @ action^T + b), e on partitions ---
            # yT columns [j*BC + b] hold y[b, 2p + j] on partition p.
            yT = cpool.tile([128, 2 * BC], fp32, name="yT", tag="yT")
            for j in range(2):  # e-parity
                ps = ppool.tile([128, BC], fp32, name=f"ps{j}", tag=f"ps{j}")
                for i in range(2):  # contraction chunk over A
                    nc.tensor.matmul(
                        ps[:],
                        pk[:, (2 * i + j) * 128 : (2 * i + j + 1) * 128],  # lhsT: [K=a, M]
                        pk[:, 2 * E + i * BC : 2 * E + (i + 1) * BC],  # rhs: [K=a, N=b]
                        start=(i == 0),
                        stop=(i == 1),
                    )
                nc.scalar.activation(
                    yT[:, j * BC : (j + 1) * BC],
                    ps[:],
                    mybir.ActivationFunctionType.Relu,
                    bias=pk[:, 2 * E + 2 * BC + j : 2 * E + 2 * BC + j + 1],
                    scale=1.0,
                )

            # --- broadcast fill + store: tile t = batch row b=t ---
            # Output row r = b*E + e with e = 2p + j: partition p's two rows
            # are adjacent, so it writes one contiguous 32KB run per DMA.
            out_ap = out[:]
            for t in range(BC):
                ft = fpool.tile([128, TILE_F], fp32, name=f"ft{t}", tag="fill")
                # One fused broadcast per tile: cols {t, BC+t} of yT hold
                # y[t, 2p] and y[t, 2p+1]; replicate each across HW.
                cols = yT.rearrange("p (j b) -> p j b", j=2)[:, :, t : t + 1]  # [128,2,1]
                src = cols.broadcast_to([128, 2, HW])
                dst = ft[:].rearrange("p (j f) -> p j f", j=2)
                if t % 2 == 0:
                    nc.vector.tensor_copy(dst, src)
                else:
                    nc.scalar.activation(dst, src, mybir.ActivationFunctionType.Copy)
                dst_ap = out_ap[E * t : E * (t + 1), :].rearrange("(p j) f -> p (j f)", p=128, j=2)
                # Alternate HWDGE rings: SP ring for DVE-filled tiles, ACT
                # ring for ACT-filled tiles (same engine as the fill, so the
                # dispatch needs no cross-engine semaphore).
                (nc.sync if t % 2 == 0 else nc.scalar).dma_start(dst_ap, ft[:])

    nc.compile()
    _NC = nc
    return nc


def _in_maps(action, conv_w, conv_b):
    action = np.asarray(action, dtype=np.float32)
    wT = np.asarray(conv_w, dtype=np.float32).T  # [A, E]
    bias = np.asarray(conv_b, dtype=np.float32).reshape(E, 1)
    # lhsT(i,j)[p, m] = conv_w[2m+j, 128i+p] = wT[128i+p, 2m+j]
    w_slices = [wT[128 * i : 128 * (i + 1), j::2] for i in range(2) for j in range(2)]
    maps = []
    for c in range(NCORES):
        actT = action[c * BC : (c + 1) * BC, :].T  # [A, BC]
        packed = np.concatenate(
            [*w_slices, actT[:128], actT[128:], bias[0::2], bias[1::2]],
            axis=1,
        )
        maps.append({"packed": np.ascontiguousarray(packed)})
    return maps


def _run_spmd(in_maps, **kwargs):
    _ensure_import_path()
    from concourse.bass_utils import run_bass_kernel_spmd

    nc = _build()
    return run_bass_kernel_spmd(nc, in_maps, list(range(NCORES)), **kwargs)


_RUNNER = None


def _make_runner():
    """Persistently-jitted equivalent of bass2jax.run_bass_via_pjrt for this
    kernel (n_cores=8): run_bass_via_pjrt builds a fresh jax.jit per call
    (~25s); caching the jitted shard_map makes repeat kernel() calls fast."""
    global _RUNNER
    if _RUNNER is not None:
        return _RUNNER
    import jax
    from concourse import bass2jax, mybir

    nc = _build()
    bass2jax.install_neuronx_cc_hook()
    partition_name = nc.partition_id_tensor.name if nc.partition_id_tensor else None

    in_names, out_names, out_avals, zero_outs = [], [], [], []
    for alloc in nc.m.functions[0].allocations:
        if not isinstance(alloc, mybir.MemoryLocationSet):
            continue
        name = alloc.memorylocations[0].name
        if alloc.kind == "ExternalInput":
            if name != partition_name:
                in_names.append(name)
        elif alloc.kind == "ExternalOutput":
            shape = tuple(alloc.tensor_shape)
            dtype = mybir.dt.np(alloc.dtype)
            out_names.append(name)
            out_avals.append(jax.core.ShapedArray(shape, dtype))
            zero_outs.append(np.zeros(shape, dtype))
    n_params, n_outs = len(in_names), len(out_avals)
    all_names = in_names + out_names + ([partition_name] if partition_name else [])
    donate = tuple(range(n_params, n_params + n_outs))

    def _body(*args):
        operands = list(args)
        if partition_name is not None:
            operands.append(bass2jax.partition_id_tensor())
        outs = bass2jax._bass_exec_p.bind(
            *operands,
            out_avals=tuple(out_avals),
            in_names=tuple(all_names),
            out_names=tuple(out_names),
            lowering_input_output_aliases=(),
            sim_require_finite=True,
            sim_require_nnan=True,
            nc=nc,
        )
        return tuple(outs)

    devices = jax.devices()[:NCORES]
    mesh = bass2jax.Mesh(np.asarray(devices), ("core",))
    sharded = jax.jit(
        bass2jax.shard_map(
            _body,
            mesh=mesh,
            in_specs=(bass2jax.PartitionSpec("core"),) * (n_params + n_outs),
            out_specs=(bass2jax.PartitionSpec("core"),) * n_outs,
            check_rep=False,
        ),
        donate_argnums=donate,
        keep_unused=True,
    )

    def run(in_maps):
        concat_in = [
            np.concatenate([np.asarray(m[nm]) for m in in_maps], axis=0)
            for nm in in_names
        ]
        concat_zeros = [
            np.zeros((NCORES * z.shape[0], *z.shape[1:]), z.dtype) for z in zero_outs
        ]
        out_arrs = sharded(*concat_in, *concat_zeros)
        return [
            {
                nm: np.asarray(out_arrs[i]).reshape(NCORES, *out_avals[i].shape)[c]
                for i, nm in enumerate(out_names)
            }
            for c in range(NCORES)
        ]

    _RUNNER = run
    return run


def kernel(action, conv_w, conv_b):
    _ensure_import_path()
    results = _make_runner()(_in_maps(action, conv_w, conv_b))
    shards = [results[c]["out"].reshape(BC, E, H, W) for c in range(NCORES)]
    return np.concatenate(shards, axis=0)


# revision 26
# speedup vs baseline: 1.0052x; 1.0052x over previous
"""Trainium2 Bass kernel: ActionEmbedder (1x1 conv on spatially-tiled action).

y[b,e] = relu(sum_a action[b,a] * conv_w[e,a] + conv_b[e])
out[b,e,h,w] = y[b,e]  (broadcast over 64x64 spatial positions)

Sharding: data-parallel over batch B=128 across 8 cores (16 rows each);
conv_w/conv_b replicated. Each core computes its 16x256 y block with 4
matmuls, then broadcasts it into [16*256, 4096] rows and streams 64 MiB
to HBM — the kernel is HBM-write-bandwidth bound.
"""

import os
import sys

import numpy as np

B, A, E, H, W = 128, 256, 256, 64, 64
NCORES = 8
BC = B // NCORES  # 16 batch rows per core
HW = H * W  # 4096 spatial positions
ROWS = BC * E  # 4096 output rows per core, each HW f32 long
TILE_F = 2 * HW  # fill-tile free dim: one batch row (= 2 e-halves) per tile

# Load-skew experiment (excluding the slow SDMA engine 15's partitions from
# some stores): DISABLED. Measured on HW, partial-partition DMAs concentrate
# onto engines 0-3 (~1.5x bytes -> 280-330us busy vs ~200us median), i.e. the
# descriptor swizzle for sub-128-partition transfers does not follow the
# documented full-width engine<->partition map, and the scheme also showed a
# nondeterministic race that CoreSim cannot reproduce. Empty dict = uniform
# full-width stores only.
SKEW_TILES = {}  # tile -> donor window start (disabled)
E15_LO, E15_HI = 92, 124  # main DMA covers partitions [0:92) and [96:124)


def _e_x():
    """Donor-partition e assignment: partitions [w+8k+q) of each window hold
    e = 184 + 64k + q (k in {0,1}, q in [0:8))."""
    ex = np.full(128, 184, dtype=np.int64)
    for w in SKEW_TILES.values():
        ex[w : w + 8] = 184 + np.arange(8)
        ex[w + 8 : w + 16] = 248 + np.arange(8)
    return ex


def _ensure_import_path():
    try:
        import concourse.bass  # noqa: F401
    except ImportError:
        for p in ("/opt/trn_rl_repo", os.path.expanduser("~/.axon_site/_ro/trn_rl_repo")):
            if os.path.isdir(p) and p not in sys.path:
                sys.path.insert(0, p)
        import concourse.bass  # noqa: F401


_NC = None


def _build():
    """Build (once) the single-core SPMD Bass program."""
    global _NC
    if _NC is not None:
        return _NC
    _ensure_import_path()
    import concourse.bacc as bacc
    import concourse.mybir as mybir
    import concourse.tile as tile

    fp32 = mybir.dt.float32
    # Bacc (not plain Bass): its compile() runs generate_event_semaphores,
    # which splits multi-wait instructions into EventSemaphore + inst — the
    # TRN2 ISA allows at most one sync wait per regular instruction.
    nc = bacc.Bacc("TRN2", target_bir_lowering=False, debug=False, num_devices=NCORES)

    # All per-core inputs packed into one [128, 546] tensor (single DMA, so
    # downstream matmuls wait on a single DMA semaphore — the PE instruction
    # has very few sync-wait slots). E is permuted even/odd on the host so
    # that partition p ends up holding y[., e=2p+j] for parity j — then each
    # partition's two output rows per batch block (2p, 2p+1) are CONTIGUOUS
    # 32KB in DRAM, halving DMA descriptor count vs the identity layout.
    # Host-side layout along the free dim ((i, j) = (A-chunk, E-parity)):
    #   [(2i+j)*128 : (2i+j+1)*128)  lhsT(i,j)[p, m] = conv_w[2m+j, 128i+p]
    #   [512:528)   actT chunk0 act0[p, b] = action[b, p]
    #   [528:544)   actT chunk1 act1[p, b] = action[b, 128 + p]
    #   [544]       bias_j=0[p] = conv_b[2p]
    #   [545]       bias_j=1[p] = conv_b[2p + 1]
    #   [546:674)   lhsT_x chunk0 [p, m] = conv_w[e_x(m), p]        (donor rows)
    #   [674:802)   lhsT_x chunk1 [p, m] = conv_w[e_x(m), 128 + p]
    #   [802]       bias_x[m] = conv_b[e_x(m)]
    F_PACKED = 2 * E + 2 * BC + 2 + 2 * 128 + 1
    packed = nc.dram_tensor("packed", [128, F_PACKED], fp32, kind="ExternalInput")
    out = nc.dram_tensor("out", [ROWS, HW], fp32, kind="ExternalOutput")
    XOFF = 2 * E + 2 * BC + 2  # 546

    with tile.TileContext(nc) as tc:
        with (
            tc.tile_pool(name="const", bufs=1) as cpool,
            tc.tile_pool(name="psum", bufs=1, space="PSUM") as ppool,
            tc.tile_pool(name="fill", bufs=4) as fpool,
        ):
            pk = cpool.tile([128, F_PACKED], fp32, name="pk", tag="pk")
            nc.sync.dma_start(pk[:], packed[:])

            # --- yT[e,b] = relu(w @ action^T + b), e on partitions ---
            # yT columns [j*BC + b] hold y[b, 2p + j] on partition p.
            yT = cpool.tile([128, 2 * BC], fp32, name="yT", tag="yT")
            for j in range(2):  # e-parity
                ps = ppool.tile([128, BC], fp32, name=f"ps{j}", tag=f"ps{j}")
                for i in range(2):  # contraction chunk over A
                    nc.tensor.matmul(
                        ps[:],
                        pk[:, (2 * i + j) * 128 : (2 * i + j + 1) * 128],  # lhsT: [K=a, M]
                        pk[:, 2 * E + i * BC : 2 * E + (i + 1) * BC],  # rhs: [K=a, N=b]
                        start=(i == 0),
                        stop=(i == 1),
                    )
                nc.scalar.activation(
                    yT[:, j * BC : (j + 1) * BC],
                    ps[:],
                    mybir.ActivationFunctionType.Relu,
                    bias=pk[:, 2 * E + 2 * BC + j : 2 * E + 2 * BC + j + 1],
                    scale=1.0,
                )

            # Donor variant: yT_x[m, b] = relu(y[b, e_x(m)]) for the aux rows.
            yTx = cpool.tile([128, BC], fp32, name="yTx", tag="yTx")
            psx = ppool.tile([128, BC], fp32, name="psx", tag="psx")
            for i in range(2):
                nc.tensor.matmul(
                    psx[:],
                    pk[:, XOFF + i * 128 : XOFF + (i + 1) * 128],
                    pk[:, 2 * E + i * BC : 2 * E + (i + 1) * BC],
                    start=(i == 0),
                    stop=(i == 1),
                )
            nc.scalar.activation(
                yTx[:],
                psx[:],
                mybir.ActivationFunctionType.Relu,
                bias=pk[:, XOFF + 256 : XOFF + 257],
                scale=1.0,
            )

            # --- broadcast fill + store: tile t = batch row b=t ---
            # Output row r = b*E + e with e = 2p + j: partition p's two rows
            # are adjacent, so it writes one contiguous 32KB run per DMA.
            out_ap = out[:]
            for t in range(BC):
                ft = fpool.tile([128, TILE_F], fp32, name=f"ft{t}", tag="fill")
                # One fused broadcast per tile: cols {t, BC+t} of yT hold
                # y[t, 2p] and y[t, 2p+1]; replicate each across HW.
                cols = yT.rearrange("p (j b) -> p j b", j=2)[:, :, t : t + 1]  # [128,2,1]
                src = cols.broadcast_to([128, 2, HW])
                dst = ft[:].rearrange("p (j f) -> p j f", j=2)
                if t % 2 == 0:
                    nc.vector.tensor_copy(dst, src)
                else:
                    nc.scalar.activation(dst, src, mybir.ActivationFunctionType.Copy)
                base = E * t
                if t not in SKEW_TILES:
                    dst_ap = out_ap[base : base + E, :].rearrange("(p j) f -> p (j f)", p=128, j=2)
                    # Alternate HWDGE rings: SP ring for DVE-filled tiles, ACT
                    # ring for ACT-filled tiles (same engine as the fill, so
                    # the dispatch needs no cross-engine semaphore).
                    (nc.sync if t % 2 == 0 else nc.scalar).dma_start(dst_ap, ft[:])
                    continue
                # Skew tile: main store skips engine-15 partitions; their 16
                # rows come from donor partitions [w:w+16) of the aux tile.
                w = SKEW_TILES[t]
                eng_a, eng_b = (nc.sync, nc.scalar) if t % 2 == 0 else (nc.scalar, nc.sync)
                dst_a = out_ap[base : base + 2 * E15_LO, :].rearrange(
                    "(p j) f -> p (j f)", p=E15_LO, j=2
                )
                eng_a.dma_start(dst_a, ft[:E15_LO, :])
                dst_b = out_ap[base + 192 : base + 2 * E15_HI, :].rearrange(
                    "(p j) f -> p (j f)", p=E15_HI - 96, j=2
                )
                eng_b.dma_start(dst_b, ft[96:E15_HI, :])
                # Donor fill: one 32-wide broadcast (allowed partition base),
                # of which [w:w+16) carries rows e=184+64k+q for tile t.
                aux = fpool.tile([128, HW], fp32, name=f"aux{t}", tag="aux", bufs=2)
                acol = yTx[w : w + 32, t : t + 1].broadcast_to([32, HW])
                nc.vector.tensor_copy(aux[w : w + 32, :], acol)
                dst_x = (
                    out_ap[base + 184 : base + E, :]
                    .rearrange("(x y) f -> x y f", x=9, y=8)[0:9:8, :, :]
                )
                eng_b.dma_start(dst_x, aux[w : w + 16, :])

    nc.compile()
    _NC = nc
    return nc


def _in_maps(action, conv_w, conv_b):
    action = np.asarray(action, dtype=np.float32)
    wT = np.asarray(conv_w, dtype=np.float32).T  # [A, E]
    bias = np.asarray(conv_b, dtype=np.float32).reshape(E, 1)
    # lhsT(i,j)[p, m] = conv_w[2m+j, 128i+p] = wT[128i+p, 2m+j]
    w_slices = [wT[128 * i : 128 * (i + 1), j::2] for i in range(2) for j in range(2)]
    ex = _e_x()
    wx_slices = [wT[128 * i : 128 * (i + 1), :][:, ex] for i in range(2)]
    bias_x = bias[ex, :]
    maps = []
    for c in range(NCORES):
        actT = action[c * BC : (c + 1) * BC, :].T  # [A, BC]
        packed = np.concatenate(
            [*w_slices, actT[:128], actT[128:], bias[0::2], bias[1::2], *wx_slices, bias_x],
            axis=1,
        )
        maps.append({"packed": np.ascontiguousarray(packed)})
    return maps


def _run_spmd(in_maps, **kwargs):
    _ensure_import_path()
    from concourse.bass_utils import run_bass_kernel_spmd

    nc = _build()
    return run_bass_kernel_spmd(nc, in_maps, list(range(NCORES)), **kwargs)


_RUNNER = None


def _make_runner():
    """Persistently-jitted equivalent of bass2jax.run_bass_via_pjrt for this
    kernel (n_cores=8): run_bass_via_pjrt builds a fresh jax.jit per call
    (~25s); caching the jitted shard_map makes repeat kernel() calls fast."""
    global _RUNNER
    if _RUNNER is not None:
        return _RUNNER
    import jax
    from concourse import bass2jax, mybir

    nc = _build()
    bass2jax.install_neuronx_cc_hook()
    partition_name = nc.partition_id_tensor.name if nc.partition_id_tensor else None

    in_names, out_names, out_avals, zero_outs = [], [], [], []
    for alloc in nc.m.functions[0].allocations:
        if not isinstance(alloc, mybir.MemoryLocationSet):
            continue
        name = alloc.memorylocations[0].name
        if alloc.kind == "ExternalInput":
            if name != partition_name:
                in_names.append(name)
        elif alloc.kind == "ExternalOutput":
            shape = tuple(alloc.tensor_shape)
            dtype = mybir.dt.np(alloc.dtype)
            out_names.append(name)
            out_avals.append(jax.core.ShapedArray(shape, dtype))
            zero_outs.append(np.zeros(shape, dtype))
    n_params, n_outs = len(in_names), len(out_avals)
    all_names = in_names + out_names + ([partition_name] if partition_name else [])
    donate = tuple(range(n_params, n_params + n_outs))

    def _body(*args):
        operands = list(args)
        if partition_name is not None:
            operands.append(bass2jax.partition_id_tensor())
        outs = bass2jax._bass_exec_p.bind(
            *operands,
            out_avals=tuple(out_avals),
            in_names=tuple(all_names),
            out_names=tuple(out_names),
            lowering_input_output_aliases=(),
            sim_require_finite=True,
            sim_require_nnan=True,
            nc=nc,
        )
        return tuple(outs)

    devices = jax.devices()[:NCORES]
    mesh = bass2jax.Mesh(np.asarray(devices), ("core",))
    sharded = jax.jit(
        bass2jax.shard_map(
            _body,
            mesh=mesh,
            in_specs=(bass2jax.PartitionSpec("core"),) * (n_params + n_outs),
            out_specs=(bass2jax.PartitionSpec("core"),) * n_outs,
            check_rep=False,
        ),
        donate_argnums=donate,
        keep_unused=True,
    )

    def run(in_maps):
        concat_in = [
            np.concatenate([np.asarray(m[nm]) for m in in_maps], axis=0)
            for nm in in_names
        ]
        concat_zeros = [
            np.zeros((NCORES * z.shape[0], *z.shape[1:]), z.dtype) for z in zero_outs
        ]
        out_arrs = sharded(*concat_in, *concat_zeros)
        return [
            {
                nm: np.asarray(out_arrs[i]).reshape(NCORES, *out_avals[i].shape)[c]
                for i, nm in enumerate(out_names)
            }
            for c in range(NCORES)
        ]

    _RUNNER = run
    return run


def kernel(action, conv_w, conv_b):
    _ensure_import_path()
    results = _make_runner()(_in_maps(action, conv_w, conv_b))
    shards = [results[c]["out"].reshape(BC, E, H, W) for c in range(NCORES)]
    return np.concatenate(shards, axis=0)


# revision 29
# speedup vs baseline: 1.0387x; 1.0333x over previous
"""Trainium2 Bass kernel: ActionEmbedder (1x1 conv on spatially-tiled action).

y[b,e] = relu(sum_a action[b,a] * conv_w[e,a] + conv_b[e])
out[b,e,h,w] = y[b,e]  (broadcast over 64x64 spatial positions)

Sharding: data-parallel over batch B=128 across 8 cores (16 rows each);
conv_w/conv_b replicated. Each core computes its 16x256 y block with 4
matmuls, then broadcasts it into [16*256, 4096] rows and streams 64 MiB
to HBM — the kernel is HBM-write-bandwidth bound.
"""

import os
import sys

import numpy as np

B, A, E, H, W = 128, 256, 256, 64, 64
NCORES = 8
BC = B // NCORES  # 16 batch rows per core
HW = H * W  # 4096 spatial positions
ROWS = BC * E  # 4096 output rows per core, each HW f32 long
TILE_F = 2 * HW  # fill-tile free dim: one batch row (= 2 e-halves) per tile

# Load-skew experiment (excluding the slow SDMA engine 15's partitions from
# some stores): DISABLED. Measured on HW, partial-partition DMAs concentrate
# onto engines 0-3 (~1.5x bytes -> 280-330us busy vs ~200us median), i.e. the
# descriptor swizzle for sub-128-partition transfers does not follow the
# documented full-width engine<->partition map, and the scheme also showed a
# nondeterministic race that CoreSim cannot reproduce. Empty dict = uniform
# full-width stores only.
SKEW_TILES = {}  # tile -> donor window start (disabled)
E15_LO, E15_HI = 92, 124  # main DMA covers partitions [0:92) and [96:124)


def _e_x():
    """Donor-partition e assignment: partitions [w+8k+q) of each window hold
    e = 184 + 64k + q (k in {0,1}, q in [0:8))."""
    ex = np.full(128, 184, dtype=np.int64)
    for w in SKEW_TILES.values():
        ex[w : w + 8] = 184 + np.arange(8)
        ex[w + 8 : w + 16] = 248 + np.arange(8)
    return ex


def _ensure_import_path():
    try:
        import concourse.bass  # noqa: F401
    except ImportError:
        for p in ("/opt/trn_rl_repo", os.path.expanduser("~/.axon_site/_ro/trn_rl_repo")):
            if os.path.isdir(p) and p not in sys.path:
                sys.path.insert(0, p)
        import concourse.bass  # noqa: F401


_NC = None


def _build():
    """Build (once) the single-core SPMD Bass program."""
    global _NC
    if _NC is not None:
        return _NC
    _ensure_import_path()
    import concourse.bacc as bacc
    import concourse.mybir as mybir
    import concourse.tile as tile

    fp32 = mybir.dt.float32
    # Bacc (not plain Bass): its compile() runs generate_event_semaphores,
    # which splits multi-wait instructions into EventSemaphore + inst — the
    # TRN2 ISA allows at most one sync wait per regular instruction.
    nc = bacc.Bacc("TRN2", target_bir_lowering=False, debug=False, num_devices=NCORES)

    # All per-core inputs packed into one [128, 546] tensor (single DMA, so
    # downstream matmuls wait on a single DMA semaphore — the PE instruction
    # has very few sync-wait slots). E is permuted even/odd on the host so
    # that partition p ends up holding y[., e=2p+j] for parity j — then each
    # partition's two output rows per batch block (2p, 2p+1) are CONTIGUOUS
    # 32KB in DRAM, halving DMA descriptor count vs the identity layout.
    # Host-side layout along the free dim ((i, j) = (A-chunk, E-parity)):
    #   [(2i+j)*128 : (2i+j+1)*128)  lhsT(i,j)[p, m] = conv_w[2m+j, 128i+p]
    #   [512:528)   actT chunk0 act0[p, b] = action[b, p]
    #   [528:544)   actT chunk1 act1[p, b] = action[b, 128 + p]
    #   [544]       bias_j=0[p] = conv_b[2p]
    #   [545]       bias_j=1[p] = conv_b[2p + 1]
    # (with SKEW_TILES non-empty, donor-variant lhsT/bias columns follow)
    F_PACKED = 2 * E + 2 * BC + 2 + (2 * 128 + 1 if SKEW_TILES else 0)
    packed = nc.dram_tensor("packed", [128, F_PACKED], fp32, kind="ExternalInput")
    out = nc.dram_tensor("out", [ROWS, HW], fp32, kind="ExternalOutput")
    XOFF = 2 * E + 2 * BC + 2  # 546

    with tile.TileContext(nc) as tc:
        with (
            tc.tile_pool(name="const", bufs=1) as cpool,
            tc.tile_pool(name="psum", bufs=1, space="PSUM") as ppool,
            tc.tile_pool(name="fill", bufs=4) as fpool,
        ):
            pk = cpool.tile([128, F_PACKED], fp32, name="pk", tag="pk")
            nc.sync.dma_start(pk[:], packed[:])

            # --- yT[e,b] = relu(w @ action^T + b), e on partitions ---
            # yT columns [j*BC + b] hold y[b, 2p + j] on partition p.
            yT = cpool.tile([128, 2 * BC], fp32, name="yT", tag="yT")
            for j in range(2):  # e-parity
                ps = ppool.tile([128, BC], fp32, name=f"ps{j}", tag=f"ps{j}")
                for i in range(2):  # contraction chunk over A
                    nc.tensor.matmul(
                        ps[:],
                        pk[:, (2 * i + j) * 128 : (2 * i + j + 1) * 128],  # lhsT: [K=a, M]
                        pk[:, 2 * E + i * BC : 2 * E + (i + 1) * BC],  # rhs: [K=a, N=b]
                        start=(i == 0),
                        stop=(i == 1),
                    )
                nc.scalar.activation(
                    yT[:, j * BC : (j + 1) * BC],
                    ps[:],
                    mybir.ActivationFunctionType.Relu,
                    bias=pk[:, 2 * E + 2 * BC + j : 2 * E + 2 * BC + j + 1],
                    scale=1.0,
                )

            # Donor variant yT_x[m, b] = relu(y[b, e_x(m)]) for the aux rows
            # of the skew scheme (disabled when SKEW_TILES is empty).
            if SKEW_TILES:
                yTx = cpool.tile([128, BC], fp32, name="yTx", tag="yTx")
                psx = ppool.tile([128, BC], fp32, name="psx", tag="psx")
                for i in range(2):
                    nc.tensor.matmul(
                        psx[:],
                        pk[:, XOFF + i * 128 : XOFF + (i + 1) * 128],
                        pk[:, 2 * E + i * BC : 2 * E + (i + 1) * BC],
                        start=(i == 0),
                        stop=(i == 1),
                    )
                nc.scalar.activation(
                    yTx[:],
                    psx[:],
                    mybir.ActivationFunctionType.Relu,
                    bias=pk[:, XOFF + 256 : XOFF + 257],
                    scale=1.0,
                )

            # --- broadcast fill + store: tile t = batch row b=t ---
            # Output row r = b*E + e with e = 2p + j: partition p's two rows
            # are adjacent, so it writes one contiguous 32KB run per DMA.
            out_ap = out[:]
            for t in range(BC):
                ft = fpool.tile([128, TILE_F], fp32, name=f"ft{t}", tag="fill")
                # One fused broadcast per tile: cols {t, BC+t} of yT hold
                # y[t, 2p] and y[t, 2p+1]; replicate each across HW.
                cols = yT.rearrange("p (j b) -> p j b", j=2)[:, :, t : t + 1]  # [128,2,1]
                src = cols.broadcast_to([128, 2, HW])
                dst = ft[:].rearrange("p (j f) -> p j f", j=2)
                if t % 2 == 0:
                    nc.vector.tensor_copy(dst, src)
                else:
                    nc.scalar.activation(dst, src, mybir.ActivationFunctionType.Copy)
                base = E * t
                if t not in SKEW_TILES:
                    dst_ap = out_ap[base : base + E, :].rearrange("(p j) f -> p (j f)", p=128, j=2)
                    # Alternate HWDGE rings: SP ring for DVE-filled tiles, ACT
                    # ring for ACT-filled tiles (same engine as the fill, so
                    # the dispatch needs no cross-engine semaphore).
                    (nc.sync if t % 2 == 0 else nc.scalar).dma_start(dst_ap, ft[:])
                    continue
                # Skew tile: main store skips engine-15 partitions; their 16
                # rows come from donor partitions [w:w+16) of the aux tile.
                w = SKEW_TILES[t]
                eng_a, eng_b = (nc.sync, nc.scalar) if t % 2 == 0 else (nc.scalar, nc.sync)
                dst_a = out_ap[base : base + 2 * E15_LO, :].rearrange(
                    "(p j) f -> p (j f)", p=E15_LO, j=2
                )
                eng_a.dma_start(dst_a, ft[:E15_LO, :])
                dst_b = out_ap[base + 192 : base + 2 * E15_HI, :].rearrange(
                    "(p j) f -> p (j f)", p=E15_HI - 96, j=2
                )
                eng_b.dma_start(dst_b, ft[96:E15_HI, :])
                # Donor fill: one 32-wide broadcast (allowed partition base),
                # of which [w:w+16) carries rows e=184+64k+q for tile t.
                aux = fpool.tile([128, HW], fp32, name=f"aux{t}", tag="aux", bufs=2)
                acol = yTx[w : w + 32, t : t + 1].broadcast_to([32, HW])
                nc.vector.tensor_copy(aux[w : w + 32, :], acol)
                dst_x = (
                    out_ap[base + 184 : base + E, :]
                    .rearrange("(x y) f -> x y f", x=9, y=8)[0:9:8, :, :]
                )
                eng_b.dma_start(dst_x, aux[w : w + 16, :])

    nc.compile()
    _NC = nc
    return nc


def _in_maps(action, conv_w, conv_b):
    action = np.asarray(action, dtype=np.float32)
    wT = np.asarray(conv_w, dtype=np.float32).T  # [A, E]
    bias = np.asarray(conv_b, dtype=np.float32).reshape(E, 1)
    # lhsT(i,j)[p, m] = conv_w[2m+j, 128i+p] = wT[128i+p, 2m+j]
    w_slices = [wT[128 * i : 128 * (i + 1), j::2] for i in range(2) for j in range(2)]
    parts = [*w_slices, None, None, bias[0::2], bias[1::2]]
    if SKEW_TILES:
        ex = _e_x()
        parts += [wT[:128, ex], wT[128:, ex], bias[ex, :]]
    maps = []
    for c in range(NCORES):
        actT = action[c * BC : (c + 1) * BC, :].T  # [A, BC]
        parts[4], parts[5] = actT[:128], actT[128:]
        maps.append({"packed": np.ascontiguousarray(np.concatenate(parts, axis=1))})
    return maps


def _run_spmd(in_maps, **kwargs):
    _ensure_import_path()
    from concourse.bass_utils import run_bass_kernel_spmd

    nc = _build()
    return run_bass_kernel_spmd(nc, in_maps, list(range(NCORES)), **kwargs)


_RUNNER = None


def _make_runner():
    """Persistently-jitted equivalent of bass2jax.run_bass_via_pjrt for this
    kernel (n_cores=8): run_bass_via_pjrt builds a fresh jax.jit per call
    (~25s); caching the jitted shard_map makes repeat kernel() calls fast."""
    global _RUNNER
    if _RUNNER is not None:
        return _RUNNER
    import jax
    from concourse import bass2jax, mybir

    nc = _build()
    bass2jax.install_neuronx_cc_hook()
    partition_name = nc.partition_id_tensor.name if nc.partition_id_tensor else None

    in_names, out_names, out_avals, zero_outs = [], [], [], []
    for alloc in nc.m.functions[0].allocations:
        if not isinstance(alloc, mybir.MemoryLocationSet):
            continue
        name = alloc.memorylocations[0].name
        if alloc.kind == "ExternalInput":
            if name != partition_name:
                in_names.append(name)
        elif alloc.kind == "ExternalOutput":
            shape = tuple(alloc.tensor_shape)
            dtype = mybir.dt.np(alloc.dtype)
            out_names.append(name)
            out_avals.append(jax.core.ShapedArray(shape, dtype))
            zero_outs.append(np.zeros(shape, dtype))
    n_params, n_outs = len(in_names), len(out_avals)
    all_names = in_names + out_names + ([partition_name] if partition_name else [])
    donate = tuple(range(n_params, n_params + n_outs))

    def _body(*args):
        operands = list(args)
        if partition_name is not None:
            operands.append(bass2jax.partition_id_tensor())
        outs = bass2jax._bass_exec_p.bind(
            *operands,
            out_avals=tuple(out_avals),
            in_names=tuple(all_names),
            out_names=tuple(out_names),
            lowering_input_output_aliases=(),
            sim_require_finite=True,
            sim_require_nnan=True,
            nc=nc,
        )
        return tuple(outs)

    devices = jax.devices()[:NCORES]
    mesh = bass2jax.Mesh(np.asarray(devices), ("core",))
    sharded = jax.jit(
        bass2jax.shard_map(
            _body,
            mesh=mesh,
            in_specs=(bass2jax.PartitionSpec("core"),) * (n_params + n_outs),
            out_specs=(bass2jax.PartitionSpec("core"),) * n_outs,
            check_rep=False,
        ),
        donate_argnums=donate,
        keep_unused=True,
    )

    def run(in_maps):
        concat_in = [
            np.concatenate([np.asarray(m[nm]) for m in in_maps], axis=0)
            for nm in in_names
        ]
        concat_zeros = [
            np.zeros((NCORES * z.shape[0], *z.shape[1:]), z.dtype) for z in zero_outs
        ]
        out_arrs = sharded(*concat_in, *concat_zeros)
        return [
            {
                nm: np.asarray(out_arrs[i]).reshape(NCORES, *out_avals[i].shape)[c]
                for i, nm in enumerate(out_names)
            }
            for c in range(NCORES)
        ]

    _RUNNER = run
    return run


def kernel(action, conv_w, conv_b):
    _ensure_import_path()
    results = _make_runner()(_in_maps(action, conv_w, conv_b))
    shards = [results[c]["out"].reshape(BC, E, H, W) for c in range(NCORES)]
    return np.concatenate(shards, axis=0)
